# revision 4
# baseline (speedup 1.0000x reference)
"""Trainium2 Bass kernel for nn_DendriteOutput.

Math: out[b, o] = sum_{d<32} x[b, o*32+d] * weight[o, o*32+d] + bias[o]
(block-diagonal connectivity: only the diagonal 32-wide blocks of `weight`
are touched, so the kernel never reads the other 99.2% of the matrix).

Sharding (8 cores, tensor-parallel over out_dim):
  core k handles outputs [k*256, (k+1)*256) for the full batch, i.e. the
  x column-slab [:, k*8192:(k+1)*8192] (32 MB/core -> the dominant HBM
  traffic; per-core roofline ~ 33 MB / ~358 GB/s ~ 94 us; measured pure-DMA
  ceiling in this container ~ 97.5 us).

Host-side layout trick: each core's x columns are permuted to d-major order
(position d*256 + o holds feature o*32 + d) and the diagonal weight strip is
shipped pre-transposed as a tiny extra input wd[1, 8192]. With that layout
the 32-way segmented reduction becomes five *fully contiguous* half-tile
adds, which DVE runs in 2x_1p perf mode; the strided o-major tree runs at 1x
and is ~2x slower (measured).

Per-core pipeline (batch tiles of 128 rows = SBUF partitions):
  1. HWDGE DMA x tile [128, 8192] f32 (contiguous rows, 4 MB).
  2. One fused scalar_tensor_tensor: xm_fp16 = (xt_f32 * 1.0) * wrep_fp16
     (cast + weight multiply in a single DVE instruction, ~5.4 us/tile).
  3. Contiguous fp16 halving tree 8192->256 (2x mode), last add in f32,
     then + bias.
  4. Store [128, 256] f32 via the sync HWDGE ring (measured: routing any
     DMA through the scalar/ACT ring is ~40% slower in this container).
DVE total ~82 us/core sits under the ~97.5 us DMA ceiling -> DMA-bound.
"""

import json

import numpy as np

import concourse.bass as bass
import concourse.bass_utils as _bass_utils
import concourse.mybir as mybir
from concourse.tile import TileContext
from concourse.bass_utils import run_bass_kernel_spmd

BATCH = 1024
OUT_DIM = 2048
DPC = 32
N_CORES = 8
O_PER = OUT_DIM // N_CORES          # 256 outputs per core
F_PER = O_PER * DPC                 # 8192 features per core
BT = 128                            # batch rows per tile (SBUF partitions)
N_BT = BATCH // BT                  # 8 batch tiles per core

# ---------------------------------------------------------------------------
# Environment workarounds (in-process only; nothing on disk is modified).
#
# The walrus build in this container (a) needs --dge-levels to lower HWDGE
# DMAs with sem waits (otherwise they hit the V2 pseudo-DMA path that allows
# none) and (b) caps sync waits at ONE per instruction while Tile attaches up
# to N (e.g. the kernel-tail drain). We add the flag and rewrite the
# serialized BIR: extra waits are hoisted into preceding single-wait Drain
# carriers on the same engine (safe: a wait only moves earlier within the
# same engine-program order).
# ---------------------------------------------------------------------------

_patched = False


def _patch_walrus_flags():
    global _patched
    if _patched:
        return
    _patched = True
    orig_rc = _bass_utils.run_command

    def rc(cmd, cwd=None, **kw):
        if cmd and "walrus_driver" in str(cmd[0]):
            cmd = list(cmd)
            cmd.insert(1, "--dge-levels=io,spill_reload,scalar_dynamic_offset")
        return orig_rc(cmd, cwd=cwd, **kw)

    _bass_utils.run_command = rc


def _split_multi_waits(bir_bytes: bytes, cap: int = 1) -> bytes:
    m = json.loads(bir_bytes)
    for fn in m["functions"]:
        for blk in fn["blocks"]:
            out = []
            for inst in blk["instructions"]:
                si = inst.get("sync_info")
                waits = (si or {}).get("on_wait") or []
                if len(waits) > cap:
                    keep = waits[-cap:]
                    for j, wchunk in enumerate(waits[:-cap]):
                        out.append(
                            {
                                "debug": inst.get("debug"),
                                "engine": inst["engine"],
                                "ins": [],
                                "name": f"{inst['name']}-ws{j}",
                                "opcode": "Drain",
                                "outs": [],
                                "sync_info": {
                                    "on_update": [],
                                    "on_wait": [wchunk],
                                },
                            }
                        )
                    si["on_wait"] = keep
                out.append(inst)
            blk["instructions"] = out
    return json.dumps(m).encode()


f32 = mybir.dt.float32
f16 = mybir.dt.float16


def _emit_body(nc, tc, x, wd, b, y, rep=0):
    """Emit one full per-core kernel inside an open TileContext."""
    MUL = mybir.AluOpType.mult
    with (
        tc.tile_pool(name=f"const{rep}", bufs=1) as cpool,
        tc.tile_pool(name=f"dram{rep}", bufs=1, space="DRAM") as dpool,
        tc.tile_pool(name=f"work{rep}", bufs=3) as wpool,
        tc.tile_pool(name=f"outp{rep}", bufs=3) as opool,
    ):
        # Stage the (host-pretransposed, d-major) diagonal weight strip:
        # cast f32->fp16 in DRAM via SWDGE, then broadcast to all 128
        # partitions with a 0-stride-source HWDGE DMA. Bias likewise.
        wrep = cpool.tile([128, F_PER], f16, name=f"wrep{rep}")
        brep = cpool.tile([128, O_PER], f32, name=f"brep{rep}")
        wdc = dpool.tile([1, F_PER], f16, name=f"wdc{rep}")
        nc.gpsimd.dma_start(wdc[:], wd[0:1, :])
        nc.sync.dma_start(
            wrep[:], bass.AP(wdc.tensor, 0, [[0, 128], [1, F_PER]])
        )
        nc.sync.dma_start(brep[:], bass.AP(b, 0, [[0, 128], [1, O_PER]]))

        for i in range(N_BT):
            ot = opool.tile([128, O_PER], f32, tag="ot", name=f"ot{rep}_{i}")
            xt = wpool.tile([128, F_PER], f32, tag="xt", bufs=3,
                            name=f"xt{rep}_{i}")
            nc.sync.dma_start(xt[:], x[i * BT : (i + 1) * BT, :])
            xm = wpool.tile([128, F_PER], f16, tag="xm", bufs=1,
                            name=f"xm{rep}_{i}")
            nc.vector.scalar_tensor_tensor(xm[:], xt[:], 1.0, wrep[:],
                                           op0=MUL, op1=MUL)
            # Contiguous halving tree: 8192 -> 256 (d-major layout makes
            # every level a dense step-1 fp16 add -> DVE 2x mode).
            n = F_PER
            cur = xm
            lvl = 0
            while n > 2 * O_PER:
                n //= 2
                q = wpool.tile([128, n], f16, tag=f"q{lvl}", bufs=1,
                               name=f"q{lvl}_{rep}_{i}")
                nc.vector.tensor_add(q[:], cur[:, 0:n], cur[:, n : 2 * n])
                cur = q
                lvl += 1
            nc.vector.tensor_add(
                ot[:], cur[:, 0:O_PER], cur[:, O_PER : 2 * O_PER]
            )
            nc.vector.tensor_add(ot[:], ot[:], brep[:])
            nc.sync.dma_start(y[i * BT : (i + 1) * BT, :], ot[:])


def _build_program(n_reps=1):
    nc = bass.Bass()
    x = nc.dram_tensor("x", [BATCH, F_PER], f32, kind="ExternalInput")
    wd = nc.dram_tensor("wd", [1, F_PER], f32, kind="ExternalInput")
    b = nc.dram_tensor("b", [O_PER], f32, kind="ExternalInput")
    y = nc.dram_tensor("y", [BATCH, O_PER], f32, kind="ExternalOutput")
    for rep in range(n_reps):
        with TileContext(nc) as tc:
            _emit_body(nc, tc, x, wd, b, y, rep=rep)
    return nc


def _finalize(nc):
    data = _split_multi_waits(nc.to_json_bytes())
    nc.to_json_bytes = lambda: data
    return nc


_CACHED = None


def _get_program():
    global _CACHED
    if _CACHED is None:
        _patch_walrus_flags()
        _CACHED = _finalize(_build_program())
    return _CACHED


def _shard_inputs(x, weight, bias):
    """Core k gets the x column-slab for outputs [k*256,(k+1)*256), permuted
    to d-major order, plus the matching d-major diagonal weight strip."""
    x = np.asarray(x, dtype=np.float32)
    weight = np.asarray(weight, dtype=np.float32)
    bias = np.asarray(bias, dtype=np.float32)
    assert x.shape == (BATCH, OUT_DIM * DPC)
    assert weight.shape == (OUT_DIM, OUT_DIM * DPC)
    oidx = np.arange(O_PER)
    didx = np.arange(DPC)
    in_maps = []
    for k in range(N_CORES):
        fs = slice(k * F_PER, (k + 1) * F_PER)
        os_ = slice(k * O_PER, (k + 1) * O_PER)
        xs = np.ascontiguousarray(
            x[:, fs].reshape(BATCH, O_PER, DPC).transpose(0, 2, 1)
            .reshape(BATCH, F_PER)
        )
        ws = weight[os_, fs]
        wb = ws[oidx[:, None], oidx[:, None] * DPC + didx[None, :]]  # [o, d]
        wd = np.ascontiguousarray(wb.T.reshape(1, F_PER))            # d-major
        in_maps.append({
            "x": xs,
            "wd": wd,
            "b": np.ascontiguousarray(bias[os_]),
        })
    return in_maps


def kernel(x, weight, bias):
    nc = _get_program()
    in_maps = _shard_inputs(x, weight, bias)
    res = run_bass_kernel_spmd(nc, in_maps, list(range(N_CORES))).results
    return np.concatenate([res[k]["y"] for k in range(N_CORES)], axis=1)


if __name__ == "__main__":
    rng = np.random.default_rng(0)
    x = rng.standard_normal((BATCH, OUT_DIM * DPC), dtype=np.float32)
    w = rng.standard_normal((OUT_DIM, OUT_DIM * DPC), dtype=np.float32)
    b_ = rng.standard_normal(OUT_DIM).astype(np.float32)
    out = kernel(x, w, b_)
    xb = x.reshape(BATCH, OUT_DIM, DPC)
    wb = np.stack([w[o, o * DPC : (o + 1) * DPC] for o in range(OUT_DIM)])
    exp = np.einsum("bod,od->bo", xb, wb) + b_
    rel = np.linalg.norm(out - exp) / np.linalg.norm(exp)
    print("rel err:", rel)
